# revision 21
# baseline (speedup 1.0000x reference)
"""Trainium2 Bass kernel: Minkowski-style instance norm (segment normalize).

Math (matches the jax reference):
    cnt[b]  = #points with batch_idx == b          (clamped to >= 1)
    mean[b] = segsum(x) / cnt[b]
    var[b]  = segsum(x^2)/cnt[b] - mean[b]^2
    out     = (x - mean[seg]) * rsqrt(var[seg]+eps) * weight + bias
            = x * scale[seg] + shift[seg]
      scale = rsqrt(var+eps)*weight ; shift = bias - mean*scale

Sharding: batch_idx is sorted, so each of the B=16 instances is a contiguous
row range.  The host assigns 2 instances to each of the 8 cores and pads each
instance into a fixed 64512-row slot, zero-filled, so the device program is
fully static: no dynamic control flow, no collectives.  Zero padding
contributes 0 to both sums; the host supplies 1/cnt directly.

Device program per core (identical SPMD program, core-local data):
  Chunks are [128 partitions x 1024 floats]; each partition holds 8
  consecutive 128-channel rows -> every DMA burst is 4KB contiguous.
  Engine balance (each [128,1024] op is ~1-2us; DMA is the bottleneck so
  every other engine stays under it):
    pass 1: PE accumulates the plain sum straight off the streamed chunks
      (fp32 matmul-accumulate vs a ones vector, quarter-rate but PE is
      otherwise idle); ACT squares each chunk; GpSimd accumulates squares.
    pass 2: VEC does out = x*scale + shift (two ops per chunk).
  Loads issue on the sync-engine HWDGE ring; stores on the scalar-engine
  ring (separate FIFO, so a store stalled on compute never blocks loads);
  tiny param DMAs on the gpsimd SWDGE ring.  The first 14 chunks of each
  instance stay resident in SBUF after pass 1, skipping their pass-2 reload.
  Program order runs both pass-1 sweeps before the pass-2 sweeps so the
  per-instance parameter derivation hides under streaming.
"""

import os
import sys

import numpy as np

for _p in ("/opt/trn_rl_repo", "/root/.axon_site/_ro/trn_rl_repo"):
    if os.path.isdir(_p) and _p not in sys.path:
        sys.path.insert(0, _p)
        break

import concourse.bacc as bacc
import concourse.bass as bass
import concourse.tile as tile
from concourse import mybir
from concourse.bass_utils import run_bass_kernel_spmd

N, C, B = 1_000_000, 128, 16
EPS = 1e-5
NCORES = 8
SEGS_PER_CORE = B // NCORES  # 2
P = 128
F32 = mybir.dt.float32

ROWS_PER_PART = 8            # consecutive rows per partition -> 4KB bursts
CHUNK_ROWS = P * ROWS_PER_PART          # 1024 rows per chunk
FW = ROWS_PER_PART * C                  # 1024 floats free width per chunk
CHUNKS_PER_SEG = 63
ROWS_PER_SEG = CHUNKS_PER_SEG * CHUNK_ROWS  # 64512 (mean seg ~62500, sd ~242)
CACHE_CHUNKS = 13            # chunks per instance kept SBUF-resident


def build_program(chunks_per_seg=CHUNKS_PER_SEG, cache_chunks=CACHE_CHUNKS,
                  xin_bufs=4, x2_bufs=5, sq_bufs=3):
    rows_per_seg = chunks_per_seg * CHUNK_ROWS
    rows_per_core = SEGS_PER_CORE * rows_per_seg

    # Bacc (not plain Bass): its compile() pass splits multi-waits into
    # event-semaphore instructions and moves matmul waits onto ldweights —
    # TRN2 hardware allows at most one sync wait per instruction.
    nc = bacc.Bacc("TRN2", target_bir_lowering=False, debug=False,
                   num_devices=NCORES)
    x = nc.dram_tensor("x", [rows_per_core, C], F32, kind="ExternalInput").ap()
    invn = nc.dram_tensor("invn", [1, SEGS_PER_CORE], F32,
                          kind="ExternalInput").ap()
    w = nc.dram_tensor("w", [1, C], F32, kind="ExternalInput").ap()
    bvec = nc.dram_tensor("b", [1, C], F32, kind="ExternalInput").ap()
    out = nc.dram_tensor("out", [rows_per_core, C], F32,
                         kind="ExternalOutput").ap()
    # row = a*1024 + p*8 + r ; chunk a is [128, (r c)], 4KB contiguous/part
    x_ch = x.rearrange("(a p r) c -> a p (r c)", p=P, r=ROWS_PER_PART)
    out_ch = out.rearrange("(a p r) c -> a p (r c)", p=P, r=ROWS_PER_PART)

    mult = mybir.AluOpType.mult
    add = mybir.AluOpType.add
    subtract = mybir.AluOpType.subtract

    with tile.TileContext(nc) as tc:
        with (
            tc.tile_pool(name="singles", bufs=1) as singles,
            tc.tile_pool(name="xin", bufs=xin_bufs) as xin,
            tc.tile_pool(name="cache", bufs=2 * cache_chunks) as cachep,
            tc.tile_pool(name="sqp", bufs=sq_bufs) as sqp,
            tc.tile_pool(name="x2", bufs=x2_bufs) as x2p,
            tc.tile_pool(name="accp", bufs=2) as accp,
            tc.tile_pool(name="pstats", bufs=1) as pstats,
            tc.tile_pool(name="bc", bufs=2) as bcp,  # [128,C] rows
            tc.tile_pool(name="psum", bufs=2, space="PSUM") as psum,
        ):
            ones_col = singles.tile([P, 1], F32)
            nc.vector.memset(ones_col, 1.0)
            ones_row = singles.tile([1, P], F32)
            nc.vector.memset(ones_row, 1.0)
            eps_sb = singles.tile([1, 1], F32)
            nc.vector.memset(eps_sb, EPS)
            invn_sb = singles.tile([1, SEGS_PER_CORE], F32)
            nc.gpsimd.dma_start(out=invn_sb, in_=invn)
            w_sb = singles.tile([1, C], F32)
            nc.gpsimd.dma_start(out=w_sb, in_=w)
            b_sb = singles.tile([1, C], F32)
            nc.gpsimd.dma_start(out=b_sb, in_=bvec)

            # Warm-up matmul: absorbs the cross-engine wait on the ones_col
            # memset so later matmuls carry a single sync wait each.
            warm = psum.tile([1, 1], F32, tag="warm")
            nc.tensor.matmul(out=warm[:], lhsT=ones_col[:, 0:1],
                             rhs=ones_col[:, 0:1], start=True, stop=True)

            cached = [[] for _ in range(SEGS_PER_CORE)]
            scale_bcs, shift_bcs = [], []
            seg_state = {}

            def p1_chunk(s, ch, sum_on_pe):
                # Serial accumulator chains pace a phase at their per-op
                # rate, so each is split into even/odd sub-chains (merged at
                # param time).  seg0's sum runs on the vector engine (idle
                # during phase A); seg1's rides the PE (idle in phase B).
                a0 = s * chunks_per_seg
                if ch == 0:
                    st = seg_state[s] = {"mm": 0}
                    st["acc_sq"] = [
                        accp.tile([P, FW], F32, tag=f"acc_sq_{par}",
                                  name=f"acc_sq_{par}_s{s}")
                        for par in range(2)]
                    if sum_on_pe:
                        st["ps_sum"] = psum.tile([1, 512], F32, tag="ps_sum",
                                                 name=f"ps_sum_s{s}")
                    else:
                        st["acc_x"] = [
                            accp.tile([P, FW], F32, tag=f"acc_x_{par}",
                                      name=f"acc_x_{par}_s{s}", bufs=1)
                            for par in range(2)]
                st = seg_state[s]
                if ch < cache_chunks:
                    xt = cachep.tile([P, FW], F32, tag="cache")
                    cached[s].append(xt)
                else:
                    xt = xin.tile([P, FW], F32, tag="xt")
                nc.sync.dma_start(out=xt[:], in_=x_ch[a0 + ch])
                sq = sqp.tile([P, FW], F32, tag="sq")
                nc.scalar.activation(
                    out=sq[:], in_=xt[:],
                    func=mybir.ActivationFunctionType.Square)
                if sum_on_pe:
                    nmm = 2 * chunks_per_seg
                    for half in (slice(0, 512), slice(512, 1024)):
                        nc.tensor.matmul(out=st["ps_sum"][:], lhsT=ones_col[:],
                                         rhs=xt[:, half], start=(st["mm"] == 0),
                                         stop=(st["mm"] == nmm - 1))
                        st["mm"] += 1
                else:
                    acc = st["acc_x"][ch % 2]
                    if ch < 2:
                        nc.vector.tensor_copy(out=acc[:], in_=xt[:])
                    else:
                        nc.vector.tensor_tensor(out=acc[:], in0=acc[:],
                                                in1=xt[:], op=add)
                accq = st["acc_sq"][ch % 2]
                if ch < 2:
                    nc.gpsimd.tensor_copy(out=accq[:], in_=sq[:])
                else:
                    nc.gpsimd.tensor_tensor(out=accq[:], in0=accq[:],
                                            in1=sq[:], op=add)

            def derive_params(s):
                st = seg_state[s]
                # merge the split accumulator chains, fold through the PE
                acc_sq = st["acc_sq"][0]
                nc.gpsimd.tensor_tensor(out=acc_sq[:], in0=acc_sq[:],
                                        in1=st["acc_sq"][1][:], op=add)
                ps_sq = psum.tile([1, 512], F32, tag="ps_sq")
                nc.tensor.matmul(out=ps_sq[:], lhsT=ones_col[:],
                                 rhs=acc_sq[:, 0:512], start=True, stop=False)
                nc.tensor.matmul(out=ps_sq[:], lhsT=ones_col[:],
                                 rhs=acc_sq[:, 512:1024], start=False,
                                 stop=True)
                if "ps_sum" in st:
                    ps_sum = st["ps_sum"]
                else:
                    acc_x = st["acc_x"][0]
                    nc.vector.tensor_tensor(out=acc_x[:], in0=acc_x[:],
                                            in1=st["acc_x"][1][:], op=add)
                    ps_sum = psum.tile([1, 512], F32, tag="ps_sum",
                                       name=f"ps_sum_s{s}")
                    nc.tensor.matmul(out=ps_sum[:], lhsT=ones_col[:],
                                     rhs=acc_x[:, 0:512], start=True,
                                     stop=False)
                    nc.tensor.matmul(out=ps_sum[:], lhsT=ones_col[:],
                                     rhs=acc_x[:, 512:1024], start=False,
                                     stop=True)

                sums_all = pstats.tile([1, 512], F32, tag="sums_all")
                nc.vector.tensor_copy(out=sums_all[:], in_=ps_sum[:])
                sqs_all = pstats.tile([1, 512], F32, tag="sqs_all")
                nc.vector.tensor_copy(out=sqs_all[:], in_=ps_sq[:])

                def _fold(row):
                    width = 512
                    while width > C:
                        half = width // 2
                        nc.vector.tensor_tensor(
                            out=row[:, :half], in0=row[:, :half],
                            in1=row[:, half:width], op=add)
                        width = half
                    return row[:, :C]

                sums = _fold(sums_all)
                sqs = _fold(sqs_all)

                mean = pstats.tile([1, C], F32, tag="mean")
                nc.vector.tensor_scalar_mul(out=mean[:], in0=sums,
                                            scalar1=invn_sb[:, s:s + 1])
                var = pstats.tile([1, C], F32, tag="var")
                nc.vector.tensor_scalar_mul(out=var[:], in0=sqs,
                                            scalar1=invn_sb[:, s:s + 1])
                meansq = pstats.tile([1, C], F32, tag="meansq")
                nc.vector.tensor_tensor(out=meansq[:], in0=mean[:], in1=mean[:],
                                        op=mult)
                nc.vector.tensor_tensor(out=var[:], in0=var[:], in1=meansq[:],
                                        op=subtract)
                scale_row = pstats.tile([1, C], F32, tag="scale_row")
                nc.scalar.activation(out=scale_row[:], in_=var[:],
                                     func=mybir.ActivationFunctionType.Sqrt,
                                     bias=eps_sb[:])
                nc.vector.reciprocal(out=scale_row[:], in_=scale_row[:])
                nc.vector.tensor_tensor(out=scale_row[:], in0=scale_row[:],
                                        in1=w_sb[:], op=mult)
                shift_row = pstats.tile([1, C], F32, tag="shift_row")
                nc.vector.tensor_tensor(out=shift_row[:], in0=mean[:],
                                        in1=scale_row[:], op=mult)
                nc.vector.tensor_tensor(out=shift_row[:], in0=b_sb[:],
                                        in1=shift_row[:], op=subtract)

                # broadcast [1, C] -> [128, C]: outer product with a ones
                # column on the PE (no DMA, no DRAM round trip), then one
                # vector copy PSUM -> SBUF
                scale_bc = bcp.tile([P, C], F32, tag="scale_bc")
                shift_bc = bcp.tile([P, C], F32, tag="shift_bc")
                for row, bc_t in ((scale_row, scale_bc), (shift_row, shift_bc)):
                    ps_bc = psum.tile([P, C], F32, tag="ps_bc",
                                      name=f"ps_bc_{row.name}")
                    nc.tensor.matmul(out=ps_bc[:], lhsT=ones_row[:],
                                     rhs=row[:], start=True, stop=True)
                    nc.vector.tensor_copy(out=bc_t[:], in_=ps_bc[:])
                scale_bcs.append(scale_bc)
                shift_bcs.append(shift_bc)

            def p2_chunk(s, ch):
                a0 = s * chunks_per_seg
                scale_bc, shift_bc = scale_bcs[s], shift_bcs[s]
                if ch < cache_chunks:
                    xt = cached[s][ch]
                else:
                    xt = x2p.tile([P, FW], F32, tag="x2")
                    nc.sync.dma_start(out=xt[:], in_=x_ch[a0 + ch])
                xt3 = xt[:].rearrange("p (a c) -> p a c", c=C)
                for bc_t, op in ((scale_bc, mult), (shift_bc, add)):
                    bc3 = bc_t[:].rearrange("p (a c) -> p a c", a=1)
                    nc.vector.tensor_tensor(out=xt3, in0=xt3,
                                            in1=bc3.to_broadcast(
                                                [P, ROWS_PER_PART, C]), op=op)
                # stores ride the scalar-engine HWDGE ring so a store
                # stalled on VEC never head-of-line blocks a load
                nc.scalar.dma_start(out=out_ch[a0 + ch], in_=xt[:])

            # Three-phase software pipeline:
            #  A: pass1(seg0) with square-accumulate on the (otherwise idle)
            #     vector engine;
            #  B: pass1(seg1) (squares accumulate on gpsimd) interleaved with
            #     pass2(seg0) (vector) — every engine and both DMA
            #     directions stay busy;
            #  C: pass2(seg1).
            # A: pass1(seg0), DMA-bound (PE idle; chains split across
            #    VEC/GPS).  B: pass1(seg1) (sum on PE) interleaved 3:2 with
            #    pass2(seg0).  After params(seg1), remaining pass-2 chunks of
            #    both segments alternate so VEC and the DMA rings never
            #    drain between phases.
            for ch in range(chunks_per_seg):
                p1_chunk(0, ch, sum_on_pe=False)
            derive_params(0)
            i1 = i2 = i3 = 0
            while i1 < chunks_per_seg:
                for _ in range(3):
                    if i1 < chunks_per_seg:
                        p1_chunk(1, i1, sum_on_pe=True)
                        i1 += 1
                for _ in range(2):
                    if i2 < chunks_per_seg:
                        p2_chunk(0, i2)
                        i2 += 1
            derive_params(1)
            while i2 < chunks_per_seg or i3 < chunks_per_seg:
                if i2 < chunks_per_seg:
                    p2_chunk(0, i2)
                    i2 += 1
                if i3 < chunks_per_seg:
                    p2_chunk(1, i3)
                    i3 += 1
    nc.compile()
    return nc


_PROGRAM = None


def _get_program():
    global _PROGRAM
    if _PROGRAM is None:
        _PROGRAM = build_program()
    return _PROGRAM


def _shard(x, batch_idx, weight, bias, rows_per_seg):
    bounds = np.searchsorted(batch_idx, np.arange(B + 1)).astype(np.int64)
    counts = np.diff(bounds)
    if counts.max() > rows_per_seg:
        raise ValueError(f"segment of {counts.max()} rows exceeds the static "
                         f"{rows_per_seg}-row slot")
    rows_per_core = SEGS_PER_CORE * rows_per_seg
    in_maps = []
    for c in range(NCORES):
        xc = np.zeros((rows_per_core, C), np.float32)
        invn = np.empty((1, SEGS_PER_CORE), np.float32)
        for s in range(SEGS_PER_CORE):
            bseg = SEGS_PER_CORE * c + s
            n = int(counts[bseg])
            xc[s * rows_per_seg:s * rows_per_seg + n] = \
                x[bounds[bseg]:bounds[bseg + 1]]
            invn[0, s] = 1.0 / max(n, 1)
        in_maps.append({"x": xc, "invn": invn, "w": weight, "b": bias})
    return in_maps, bounds, counts


def _gather(results, bounds, counts, rows_per_seg):
    y = np.empty((N, C), np.float32)
    for c in range(NCORES):
        oc = results[c]["out"]
        for s in range(SEGS_PER_CORE):
            bseg = SEGS_PER_CORE * c + s
            n = int(counts[bseg])
            y[bounds[bseg]:bounds[bseg + 1]] = \
                oc[s * rows_per_seg:s * rows_per_seg + n]
    return y


def kernel(x, batch_idx, weight, bias, trace=False, trace_dir=None):
    x = np.ascontiguousarray(np.asarray(x, dtype=np.float32))
    batch_idx = np.asarray(batch_idx)
    weight = np.ascontiguousarray(np.asarray(weight, dtype=np.float32)).reshape(1, C)
    bias = np.ascontiguousarray(np.asarray(bias, dtype=np.float32)).reshape(1, C)

    in_maps, bounds, counts = _shard(x, batch_idx, weight, bias, ROWS_PER_SEG)
    nc = _get_program()
    res = run_bass_kernel_spmd(nc, in_maps, list(range(NCORES)), trace=trace,
                               tmpdir=trace_dir)
    y = _gather(res.results, bounds, counts, ROWS_PER_SEG)
    if trace:
        return y, res
    return y
